# revision 18
# baseline (speedup 1.0000x reference)
"""MoE routing kernel for Trainium2 (8 NeuronCores, expert-parallel).

Computes, for full inputs x[B,S,D], Wg[D,E], W1[E,D,H], b1[E,H], W2[E,H,D],
b2[E,D]:
    y = moe_forward(x, ...)            # GShard/tutel style top-2 capacity MoE
    out = log_softmax(sum(y, axis=D), axis=S)   # [B, S]

Key algebraic simplification: only row-sums of the second expert matmul are
needed, so  sum_d(yexp[e,c,:]) = relu(x W1[e] + b1[e]) . rowsum(W2[e]) +
sum(b2[e]).  The rowsum of W2 is computed on device.

Distribution: expert-parallel (core r owns expert r), data-parallel router,
replicated position scan, AllGather for logits, AllReduce for the final
per-token scalars.
"""

import sys

for _p in ("/opt/trn_rl_repo",):
    if _p not in sys.path:
        sys.path.insert(0, _p)

import numpy as np
from dataclasses import dataclass

import concourse.bass as bass
import concourse.tile as tile
from concourse import bacc, mybir, library_config
from concourse.bass_utils import run_bass_kernel_spmd

AF = mybir.ActivationFunctionType
OP = mybir.AluOpType
F32 = mybir.dt.float32
F32R = mybir.dt.float32r
I16 = mybir.dt.int16
U16 = mybir.dt.uint16
I32 = mybir.dt.int32
U32 = mybir.dt.uint32


@dataclass(frozen=True)
class Cfg:
    B: int = 4
    S: int = 2048
    D: int = 2048
    H: int = 2048
    E: int = 8
    K: int = 2          # top-k
    NC: int = 8         # cores

    @property
    def T(self):
        return self.B * self.S

    @property
    def C(self):
        return (self.K * self.T) // self.E

    @property
    def TL(self):  # tokens per core (router data-parallel)
        return self.T // self.NC

    @property
    def DT(self):
        return self.D // 128

    @property
    def HT(self):
        return self.H // 128

    @property
    def J(self):  # number of 128-slot tiles in the k*T priority sequence
        return (self.K * self.T) // 128

    @property
    def JT(self):  # number of 128-token tiles
        return self.T // 128

    @property
    def THI(self):  # t_hi factor for the combine scatter (t = t_hi*128 + t_lo)
        return self.T // 128

    @property
    def R(self):  # c = rr*16 + q factorization, rr in [0, R)
        return self.C // 16

    @property
    def CCH(self):  # dispatch gather chunk (slots)
        return min(256, self.C)

    @property
    def NCC(self):
        return self.C // self.CCH

    @property
    def LCH(self):  # logits gather chunk (tokens)
        return min(256, self.TL)

    @property
    def NLC(self):
        return self.TL // self.LCH


def _ckd(cfg: Cfg):
    assert cfg.T % 128 == 0 and cfg.D % 128 == 0 and cfg.H % 128 == 0
    assert cfg.K == 2 and cfg.E <= 8
    assert cfg.J <= 128 and cfg.R <= 128 and cfg.THI <= 128
    assert cfg.C % 16 == 0 and cfg.TL % cfg.LCH == 0
    assert cfg.D % 256 == 0  # dma_gather elem_size_bytes % 256


def build_program(cfg: Cfg, debug_outputs: bool = False, finalize: bool = True):
    """Build the SPMD bass program (same program for all 8 cores)."""
    _ckd(cfg)
    nc = bacc.Bacc("TRN2", target_bir_lowering=False, debug=False,
                   num_devices=cfg.NC, num_swdge_queues=4)

    T, D, H, E, C = cfg.T, cfg.D, cfg.H, cfg.E, cfg.C
    DT, HT, J, R, THI = cfg.DT, cfg.HT, cfg.J, cfg.R, cfg.THI
    NPOS = J * E

    # ---------------- dram I/O ----------------
    x_lo = nc.dram_tensor("x_lo", [T, D], U16, kind="ExternalInput")
    x_hi = nc.dram_tensor("x_hi", [T, D], U16, kind="ExternalInput")
    wg_d = nc.dram_tensor("wg", [D, E], F32, kind="ExternalInput")
    w1_d = nc.dram_tensor("w1e", [D, H], F32, kind="ExternalInput")
    b1_d = nc.dram_tensor("b1e", [128, HT], F32, kind="ExternalInput")
    w2_d = nc.dram_tensor("w2e", [H, D], F32, kind="ExternalInput")
    b2_d = nc.dram_tensor("b2e", [1, D], F32, kind="ExternalInput")
    eid_d = nc.dram_tensor("eid", [128, 1], F32, kind="ExternalInput")
    tokidx_d = nc.dram_tensor("tokidx", [128, cfg.TL // 16], I16,
                              kind="ExternalInput")
    # constants
    cU_d = nc.dram_tensor("cU", [128, 128], F32, kind="ExternalInput")
    cI_d = nc.dram_tensor("cI", [128, 128], F32, kind="ExternalInput")
    cOnesK1_d = nc.dram_tensor("cOnesK1", [1, 128], F32, kind="ExternalInput")
    cOnesM1_d = nc.dram_tensor("cOnesM1", [128, 1], F32, kind="ExternalInput")
    cREP_d = nc.dram_tensor("cREP", [16, 128], F32, kind="ExternalInput")
    cIota8_d = nc.dram_tensor("cIota8", [128, 8], F32, kind="ExternalInput")
    cIota8m_d = nc.dram_tensor("cIota8m", [128, 8], F32, kind="ExternalInput")
    cIota16_d = nc.dram_tensor("cIota16", [128, 16], F32, kind="ExternalInput")
    cIota128_d = nc.dram_tensor("cIota128", [128, 128], F32, kind="ExternalInput")
    cIotaTHI_d = nc.dram_tensor("cIotaTHI", [128, THI], F32, kind="ExternalInput")
    cTval_d = nc.dram_tensor("cTval", [128, J], F32, kind="ExternalInput")

    out_d = nc.dram_tensor("out", [cfg.B, cfg.S], F32, kind="ExternalOutput")
    dbg = {}
    if debug_outputs:
        dbg["logits"] = nc.dram_tensor("dbg_logits", [128, cfg.JT * E], F32,
                                       kind="ExternalOutput")
        dbg["pos"] = nc.dram_tensor("dbg_pos", [128, J], F32,
                                    kind="ExternalOutput")
        dbg["gts"] = nc.dram_tensor("dbg_gts", [128, J], F32,
                                    kind="ExternalOutput")
        dbg["tok"] = nc.dram_tensor("dbg_tok", [16, R], F32,
                                    kind="ExternalOutput")
        dbg["gate"] = nc.dram_tensor("dbg_gate", [16, R], F32,
                                     kind="ExternalOutput")
        dbg["s"] = nc.dram_tensor("dbg_s", [1, C], F32, kind="ExternalOutput")
        dbg["ysum"] = nc.dram_tensor("dbg_ysum", [THI, 128], F32,
                                     kind="ExternalOutput")

    # collective bounce buffers
    lg_bounce = nc.dram_tensor("lg_bounce", [cfg.TL, E], F32)
    lg_all = nc.dram_tensor("lg_all", [T, E], F32, addr_space="Shared")
    y_bounce = nc.dram_tensor("y_bounce", [THI, 128], F32)
    y_all = nc.dram_tensor("y_all", [THI, 128], F32, addr_space="Shared")

    groups = [list(range(cfg.NC))]

    def r32(ap):
        return ap.bitcast(F32R)

    with tile.TileContext(nc) as tc:
        nc.gpsimd.load_library(library_config.mlp)

        # ---------- persistent pools ----------
        with tc.tile_pool(name="consts", bufs=1) as cpool, \
             tc.tile_pool(name="w1", bufs=1) as w1pool, \
             tc.tile_pool(name="small", bufs=1) as spool:

            # constants to SBUF
            cU = cpool.tile([128, 128], F32)
            cUr = cpool.tile([128, 128], F32R)
            cOnesM1r = cpool.tile([128, 1], F32R)
            cI = cpool.tile([128, 128], F32)
            cOnesK1 = cpool.tile([1, 128], F32)
            cOnesM1 = cpool.tile([128, 1], F32)
            cREP = cpool.tile([16, 128], F32)
            cIota8 = cpool.tile([128, 8], F32)
            cIota8m = cpool.tile([128, 8], F32)
            cIota16 = cpool.tile([128, 16], F32)
            cIota128 = cpool.tile([128, 128], F32)
            cIotaTHI = cpool.tile([128, THI], F32)
            cTval = cpool.tile([128, J], F32)
            nc.scalar.dma_start(cUr[:], cU_d[:].bitcast(F32R))
            nc.scalar.dma_start(cOnesM1r[:], cOnesM1_d[:].bitcast(F32R))
            for sb, dr in ((cU, cU_d), (cI, cI_d), (cOnesK1, cOnesK1_d),
                           (cOnesM1, cOnesM1_d), (cREP, cREP_d),
                           (cIota8, cIota8_d), (cIota8m, cIota8m_d),
                           (cIota16, cIota16_d), (cIota128, cIota128_d),
                           (cIotaTHI, cIotaTHI_d), (cTval, cTval_d)):
                nc.scalar.dma_start(sb[:], dr[:])

            eid = cpool.tile([128, 1], F32)
            nc.scalar.dma_start(eid[:], eid_d[:])
            tokidx = cpool.tile([128, cfg.TL // 16], I16)
            nc.scalar.dma_start(tokidx[:], tokidx_d[:])

            wg_sb = cpool.tile([128, DT, E], F32R)
            nc.scalar.dma_start(
                wg_sb[:],
                wg_d.rearrange("(dt p) e -> p dt e", p=128).bitcast(F32R))
            b1_sb = cpool.tile([128, HT], F32)
            nc.scalar.dma_start(b1_sb[:], b1_d[:])

            # W1 resident: [p, dt, h]
            w1_sb = w1pool.tile([128, DT, H], F32R)
            for dt in range(DT):
                nc.sync.dma_start(w1_sb[:, dt, :],
                                  w1_d[dt * 128:(dt + 1) * 128, :].bitcast(F32R))

            # w2sum[h] = sum_d W2[h, d]  -> [p, ht]
            w2s_sb = spool.tile([128, HT], F32R)
            with tc.tile_pool(name="w2red", bufs=2) as w2pool:
                for ht in range(HT):
                    w2t = w2pool.tile([128, D], F32, tag="w2t")
                    nc.sync.dma_start(w2t[:], w2_d[ht * 128:(ht + 1) * 128, :])
                    with nc.allow_low_precision(reason="fp32r rowsum"):
                        nc.vector.tensor_reduce(w2s_sb[:, ht:ht + 1], w2t[:],
                                                mybir.AxisListType.X, OP.add)
            # b2sum scalar
            b2s = spool.tile([1, 1], F32)
            with tc.tile_pool(name="b2red", bufs=1) as b2pool:
                b2row = b2pool.tile([1, D], F32)
                nc.scalar.dma_start(b2row[:], b2_d[:])
                nc.vector.tensor_reduce(b2s[:], b2row[:],
                                        mybir.AxisListType.X, OP.add)

            # ================= P1: router logits (my TL tokens) ============
            # lgT_sb[e, t_local]
            lgp0_cm = tc.tile_pool(name="lgp0", bufs=1)
            lgp0 = lgp0_cm.__enter__()
            lgT = lgp0.tile([8, cfg.TL], F32)
            with tc.tile_pool(name="lgg", bufs=2) as lgp, \
                 tc.tile_pool(name="lgps", bufs=2, space="PSUM") as lgps:
                for lc in range(cfg.NLC):
                    pl_lo = lgp.tile([128, DT, cfg.LCH], U16, tag="pl_lo")
                    pl_hi = lgp.tile([128, DT, cfg.LCH], U16, tag="pl_hi")
                    idxs = tokidx[:, (lc * cfg.LCH) // 16:
                                  ((lc + 1) * cfg.LCH) // 16]
                    nc.gpsimd.dma_gather(pl_lo[:], x_lo[:], idxs, cfg.LCH,
                                         cfg.LCH, D, transpose=True,
                                         queue_num=0)
                    nc.gpsimd.dma_gather(pl_hi[:], x_hi[:], idxs, cfg.LCH,
                                         cfg.LCH, D, transpose=True,
                                         queue_num=1)
                    # recombine fp32 = (hi << 16) | lo
                    xt = lgp.tile([128, DT, cfg.LCH], F32R, tag="xt")
                    for dt in range(DT):
                        stg = lgp.tile([128, cfg.LCH], U32, tag="stg",
                                       bufs=2)
                        nc.vector.scalar_tensor_tensor(
                            stg[:], pl_hi[:, dt, :], 65536, pl_lo[:, dt, :],
                            OP.mult, OP.add)
                        nc.vector.tensor_copy(xt[:, dt, :],
                                              stg.bitcast(F32)[:])
                    ps = lgps.tile([8, cfg.LCH], F32, tag="lgps")
                    for dt in range(DT):
                        nc.tensor.matmul(ps[:], wg_sb[:, dt, :],
                                         xt[:, dt, :],
                                         start=(dt == 0), stop=(dt == DT - 1))
                    nc.scalar.copy(lgT[:, lc * cfg.LCH:(lc + 1) * cfg.LCH],
                                   ps[:])

            # transpose to [t_local % 128, jj, e] then DMA out + AllGather
            lg_me = lgp0.tile([128, cfg.TL // 128, 8], F32)
            with tc.tile_pool(name="lgtp", bufs=2, space="PSUM") as lgtp:
                for m in range(cfg.TL // 128):
                    pst = lgtp.tile([128, 8], F32, tag="pst")
                    nc.tensor.transpose(pst[:], lgT[0:8, m * 128:(m + 1) * 128],
                                        cI[0:8, 0:8])
                    nc.scalar.copy(lg_me[:, m, :], pst[:])
            nc.sync.dma_start(
                lg_bounce.rearrange("(m p) e -> p m e", p=128), lg_me[:])
            nc.gpsimd.collective_compute(
                "AllGather", OP.bypass, replica_groups=groups,
                ins=[lg_bounce[:]], outs=[lg_all[:]])
            lgp0_cm.__exit__(None, None, None)

            rtp_cm = tc.tile_pool(name="rtp", bufs=1)
            rtp = rtp_cm.__enter__()
            # LG[p, jt, e] for all T tokens
            LG = rtp.tile([128, cfg.JT, E], F32)
            nc.sync.dma_start(LG[:],
                                lg_all.rearrange("(jt p) e -> p jt e", p=128))
            if debug_outputs:
                nc.sync.dma_start(dbg["logits"][:],
                                    LG[:].rearrange("p a b -> p (a b)"))

            # ================= P2: top-2 + gates ============
            JT = cfg.JT
            m1 = rtp.tile([128, JT], F32)
            nc.vector.tensor_reduce(m1[:], LG[:], mybir.AxisListType.X, OP.max)
            eq1 = rtp.tile([128, JT, E], F32)
            nc.vector.tensor_tensor(
                eq1[:], LG[:],
                m1[:].unsqueeze(2).broadcast_to([128, JT, E]),
                OP.is_equal)
            zt = rtp.tile([128, JT, E], F32)
            nc.vector.tensor_tensor(
                zt[:], eq1[:],
                cIota8m[:].unsqueeze(1).broadcast_to([128, JT, E]),
                OP.mult)
            idx_all = rtp.tile([128, J], F32)
            i1m = rtp.tile([128, JT], F32)
            nc.vector.tensor_reduce(i1m[:], zt[:], mybir.AxisListType.X, OP.min)
            nc.vector.tensor_scalar(idx_all[:, 0:JT], i1m[:], 999.0, None,
                                    OP.add)
            masked = rtp.tile([128, JT, E], F32)
            nc.vector.scalar_tensor_tensor(masked[:], eq1[:], -1e30, LG[:],
                                           OP.mult, OP.add)
            m2 = rtp.tile([128, JT], F32)
            nc.vector.tensor_reduce(m2[:], masked[:], mybir.AxisListType.X,
                                    OP.max)
            eq2 = rtp.tile([128, JT, E], F32)
            nc.vector.tensor_tensor(
                eq2[:], masked[:],
                m2[:].unsqueeze(2).broadcast_to([128, JT, E]),
                OP.is_equal)
            zt2 = rtp.tile([128, JT, E], F32)
            nc.vector.tensor_tensor(
                zt2[:], eq2[:],
                cIota8m[:].unsqueeze(1).broadcast_to([128, JT, E]),
                OP.mult)
            i2m = rtp.tile([128, JT], F32)
            nc.vector.tensor_reduce(i2m[:], zt2[:], mybir.AxisListType.X,
                                    OP.min)
            nc.vector.tensor_scalar(idx_all[:, JT:J], i2m[:], 999.0, None,
                                    OP.add)
            # gates: g1 = sigmoid(m1 - m2), g2 = 1 - g1
            gts = rtp.tile([128, J], F32)
            d12 = rtp.tile([128, JT], F32)
            nc.vector.tensor_tensor(d12[:], m1[:], m2[:], OP.subtract)
            nc.scalar.activation(gts[:, 0:JT], d12[:], AF.Sigmoid)
            nc.vector.tensor_scalar(gts[:, JT:J], gts[:, 0:JT], -1.0, 1.0,
                                    OP.mult, OP.add)

            # ================= P3: position scan ============
            OH = rtp.tile([128, J, E], F32R)
            nc.vector.tensor_tensor(
                OH[:],
                idx_all[:].unsqueeze(2).broadcast_to([128, J, E]),
                cIota8[:].unsqueeze(1).broadcast_to([128, J, E]),
                OP.is_equal)
            OHf32 = OH.bitcast(F32)
            OHf = OH[:].rearrange("p a b -> p (a b)")
            nch = (NPOS + 511) // 512
            csz = NPOS // nch
            scanps_cm = tc.tile_pool(name="scanps", bufs=1, space="PSUM")
            scanps = scanps_cm.__enter__()
            ps_pos = scanps.tile([128, NPOS], F32)
            ps_tot = scanps.tile([1, NPOS], F32)
            for h in range(nch):
                sl = slice(h * csz, (h + 1) * csz)
                nc.tensor.matmul(ps_pos[:, sl], cUr[:], OHf[:, sl],
                                 start=True, stop=True)
                nc.tensor.matmul(ps_tot[:, sl], cOnesM1r[:],
                                 OHf[:, sl], start=True, stop=True)
            tot_row = rtp.tile([1, NPOS], F32)
            nc.scalar.copy(tot_row[:], ps_tot[:])
            tot_jt = rtp.tile([J, E], F32)
            nc.sync.dma_start(tot_jt[:], tot_row[:])
            ps_off = scanps.tile([J, E], F32)
            nc.tensor.matmul(ps_off[:], cU[0:J, 0:J], tot_jt[:],
                             start=True, stop=True)
            off_sb = rtp.tile([J, E], F32)
            nc.scalar.copy(off_sb[:], ps_off[:])
            off_row = rtp.tile([1, NPOS], F32)
            nc.sync.dma_start(off_row[:], off_sb[:])
            ps_bc = scanps.tile([128, NPOS], F32)
            for h in range(nch):
                sl = slice(h * csz, (h + 1) * csz)
                nc.tensor.matmul(ps_bc[:, sl], cOnesK1[:], off_row[:, sl],
                                 start=True, stop=True)
            # pos_full = intra + offset broadcast
            posf = rtp.tile([128, NPOS], F32)
            nc.scalar.copy(posf[:], ps_pos[:])
            nc.vector.tensor_tensor(posf[:], posf[:], ps_bc[:], OP.add)
            # select position of chosen expert; keep; final gates
            tmp = rtp.tile([128, J, E], F32)
            nc.vector.tensor_tensor(tmp[:].rearrange("p a b -> p (a b)"),
                                    OHf32[:].rearrange("p a b -> p (a b)"),
                                    posf[:], OP.mult)
            POS = rtp.tile([128, J], F32)
            nc.vector.tensor_reduce(POS[:], tmp[:], mybir.AxisListType.X,
                                    OP.add)
            scanps_cm.__exit__(None, None, None)
            KEEP = rtp.tile([128, J], F32)
            nc.vector.tensor_scalar(KEEP[:], POS[:], float(C), None, OP.is_lt)
            gfin = rtp.tile([128, J], F32)
            nc.vector.tensor_tensor(gfin[:], gts[:], KEEP[:], OP.mult)
            if debug_outputs:
                nc.sync.dma_start(dbg["pos"][:], POS[:])
                nc.sync.dma_start(dbg["gts"][:], gfin[:])

            # ================= P4: inversion (slot -> token, gate) =========
            SEL = rtp.tile([128, J], F32)
            nc.vector.tensor_scalar(SEL[:], idx_all[:], eid[:], None,
                                    OP.is_equal)
            selk = rtp.tile([128, J], F32)
            nc.vector.tensor_tensor(selk[:], SEL[:], KEEP[:], OP.mult)
            valt = rtp.tile([128, J], F32)
            nc.vector.tensor_tensor(valt[:], cTval[:, 0:J], selk[:], OP.mult)
            valg = rtp.tile([128, J], F32)
            nc.vector.tensor_tensor(valg[:], gfin[:], selk[:], OP.mult)
            # integer split of POS: q = POS & 15, rr = POS >> 4
            posi = rtp.tile([128, J], I32)
            nc.vector.tensor_copy(posi[:], POS[:])
            rri = rtp.tile([128, J], I32)
            nc.vector.tensor_scalar(rri[:], posi[:], 4, None,
                                    OP.logical_shift_right)
            qi = rtp.tile([128, J], I32)
            nc.vector.tensor_scalar(qi[:], posi[:], 15, None, OP.bitwise_and)
            rrf = rtp.tile([128, J], F32)
            nc.vector.tensor_copy(rrf[:], rri[:])
            qf = rtp.tile([128, J], F32)
            nc.vector.tensor_copy(qf[:], qi[:])

            invps_cm = tc.tile_pool(name="invps", bufs=1, space="PSUM")
            invps = invps_cm.__enter__()
            ps_inv = invps.tile([32, R], F32)
            STR = 4  # j's per strip
            with tc.tile_pool(name="invp", bufs=2) as invp:
                for s0 in range(0, J, STR):
                    js = slice(s0, s0 + STR)
                    aoh = invp.tile([128, STR, 16], F32, tag="aoh")
                    nc.vector.tensor_tensor(
                        aoh[:],
                        qf[:, js].unsqueeze(2).broadcast_to(
                            [128, STR, 16]),
                        cIota16[:].unsqueeze(1).broadcast_to(
                            [128, STR, 16]),
                        OP.is_equal)
                    atile = invp.tile([128, STR, 32], F32, tag="atile")
                    nc.vector.tensor_tensor(
                        atile[:, :, 0:16], aoh[:],
                        valt[:, js].unsqueeze(2).broadcast_to(
                            [128, STR, 16]),
                        OP.mult)
                    nc.vector.tensor_tensor(
                        atile[:, :, 16:32], aoh[:],
                        valg[:, js].unsqueeze(2).broadcast_to(
                            [128, STR, 16]),
                        OP.mult)
                    btile = invp.tile([128, STR, R], F32, tag="btile")
                    nc.vector.tensor_tensor(
                        btile[:],
                        rrf[:, js].unsqueeze(2).broadcast_to(
                            [128, STR, R]),
                        cIota128[:, 0:R].unsqueeze(1).broadcast_to(
                            [128, STR, R]),
                        OP.is_equal)
                    for jj in range(STR):
                        j = s0 + jj
                        nc.tensor.matmul(ps_inv[:], atile[:, jj, :],
                                         btile[:, jj, :],
                                         start=(j == 0), stop=(j == J - 1))
            rtp_cm.__exit__(None, None, None)
            inv_sb = spool.tile([32, R], F32)
            nc.scalar.copy(inv_sb[:], ps_inv[:])
            if debug_outputs:
                nc.sync.dma_start(dbg["tok"][:], inv_sb[0:16, :])
                nc.sync.dma_start(dbg["gate"][:], inv_sb[16:32, :])
            # gate map to partitions 0:16 (cross-partition -> DMA)
            ge16 = spool.tile([16, R], F32)
            nc.sync.dma_start(ge16[:], inv_sb[16:32, :])
            # replicate token map over 128 partitions, convert to int16
            ps_rep = invps.tile([128, R], F32, tag="ps_rep")
            nc.tensor.matmul(ps_rep[:], cREP[:], inv_sb[0:16, :],
                             start=True, stop=True)
            tokmap16 = spool.tile([128, R], I16)
            nc.vector.tensor_copy(tokmap16[:], ps_rep[:])
            invps_cm.__exit__(None, None, None)

            # ================= P5: dispatch gather + expert FFN ============
            sT = spool.tile([R, 16], F32)
            CCH = cfg.CCH
            with tc.tile_pool(name="ffg", bufs=1) as gpl, \
                 tc.tile_pool(name="ffx", bufs=2) as xpl, \
                 tc.tile_pool(name="ffh", bufs=2) as hpl, \
                 tc.tile_pool(name="ffps", bufs=2, space="PSUM") as fps, \
                 tc.tile_pool(name="ffss", bufs=2, space="PSUM") as sps:
                for cc in range(cfg.NCC):
                    idxs = tokmap16[:, (cc * CCH) // 16:((cc + 1) * CCH) // 16]
                    g_lo = gpl.tile([128, DT, CCH], U16, tag="g_lo")
                    g_hi = gpl.tile([128, DT, CCH], U16, tag="g_hi")
                    nc.gpsimd.dma_gather(g_lo[:], x_lo[:], idxs, CCH, CCH, D,
                                         transpose=True, queue_num=2)
                    nc.gpsimd.dma_gather(g_hi[:], x_hi[:], idxs, CCH, CCH, D,
                                         transpose=True, queue_num=3)
                    xet = xpl.tile([128, DT, CCH], F32R, tag="xet")
                    for dt in range(DT):
                        fstg = xpl.tile([128, CCH], U32, tag="fstg", bufs=2)
                        nc.vector.scalar_tensor_tensor(
                            fstg[:], g_hi[:, dt, :], 65536, g_lo[:, dt, :],
                            OP.mult, OP.add)
                        nc.vector.tensor_copy(xet[:, dt, :],
                                              fstg.bitcast(F32)[:])
                    ps_s = sps.tile([1, CCH], F32, tag="ps_s")
                    for ht in range(HT):
                        ps_h = fps.tile([128, CCH], F32, tag="ps_h")
                        for dt in range(DT):
                            nc.tensor.matmul(
                                ps_h[:],
                                w1_sb[:, dt, ht * 128:(ht + 1) * 128],
                                xet[:, dt, :],
                                start=(dt == 0), stop=(dt == DT - 1))
                        ht_sb = hpl.tile([128, CCH], F32R, tag="ht_sb")
                        nc.scalar.activation(ht_sb[:], ps_h[:], AF.Relu,
                                             bias=b1_sb[:, ht:ht + 1])
                        nc.tensor.matmul(ps_s[:], w2s_sb[:, ht:ht + 1],
                                         ht_sb[:],
                                         start=(ht == 0), stop=(ht == HT - 1))
                    s_st = hpl.tile([1, CCH], F32, tag="s_st")
                    nc.vector.tensor_scalar(s_st[:], ps_s[:], b2s[0:1, 0:1],
                                            None, OP.add)
                    nc.scalar.dma_start(
                        sT[(cc * CCH) // 16:((cc + 1) * CCH) // 16, :],
                        s_st[:])
            if debug_outputs:
                nc.sync.dma_start(dbg["s"][:], sT[:])

            # ================= P6: combine scatter ============
            combps_cm = tc.tile_pool(name="combps", bufs=1, space="PSUM")
            combps = combps_cm.__enter__()
            ps_geT = combps.tile([128, 16], F32, tag="ps_geT")
            nc.tensor.transpose(ps_geT[0:R, :], ge16[:], cI[0:16, 0:16])
            wT = spool.tile([R, 16], F32)
            nc.vector.tensor_tensor(wT[:], sT[:], ps_geT[0:R, :], OP.mult)
            ps_tT = combps.tile([128, 16], F32, tag="ps_tT")
            nc.tensor.transpose(ps_tT[0:R, :], inv_sb[0:16, :], cI[0:16, 0:16])
            tokT = spool.tile([R, 16], F32)
            nc.scalar.copy(tokT[:], ps_tT[0:R, :])
            # integer split: thi = tok >> 7, tlo = tok & 127
            tki = spool.tile([R, 16], I32)
            nc.vector.tensor_copy(tki[:], tokT[:])
            thi_i = spool.tile([R, 16], I32)
            nc.vector.tensor_scalar(thi_i[:], tki[:], 7, None,
                                    OP.logical_shift_right)
            tlo_i = spool.tile([R, 16], I32)
            nc.vector.tensor_scalar(tlo_i[:], tki[:], 127, None,
                                    OP.bitwise_and)
            thi_f = spool.tile([R, 16], F32)
            nc.vector.tensor_copy(thi_f[:], thi_i[:])
            tlo_f = spool.tile([R, 16], F32)
            nc.vector.tensor_copy(tlo_f[:], tlo_i[:])

            ps_y = combps.tile([THI, 128], F32)
            with tc.tile_pool(name="scat", bufs=2) as scp:
                for j in range(16):
                    a_j = scp.tile([R, THI], F32, tag="a_j")
                    nc.vector.scalar_tensor_tensor(
                        a_j[:], cIotaTHI[0:R, :], thi_f[:, j:j + 1],
                        wT[:, j:j + 1].broadcast_to([R, THI]),
                        OP.is_equal, OP.mult)
                    b_j = scp.tile([R, 128], F32, tag="b_j")
                    nc.vector.tensor_scalar(b_j[:], cIota128[0:R, :],
                                            tlo_f[:, j:j + 1], None,
                                            OP.is_equal)
                    nc.tensor.matmul(ps_y[:], a_j[:], b_j[:],
                                     start=(j == 0), stop=(j == 15))
            ysum_sb = spool.tile([THI, 128], F32)
            nc.scalar.copy(ysum_sb[:], ps_y[:])
            combps_cm.__exit__(None, None, None)
            if debug_outputs:
                nc.sync.dma_start(dbg["ysum"][:], ysum_sb[:])
            nc.sync.dma_start(y_bounce[:], ysum_sb[:])
            nc.gpsimd.collective_compute(
                "AllReduce", OP.add, replica_groups=groups,
                ins=[y_bounce[:]], outs=[y_all[:]])

            # ================= P7: log_softmax over S ============
            B, S = cfg.B, cfg.S
            tailp_cm = tc.tile_pool(name="tailp", bufs=1)
            tailp = tailp_cm.__enter__()
            Y4 = tailp.tile([B, S], F32)
            nc.sync.dma_start(Y4[:],
                                y_all.rearrange("(b u) l -> b (u l)", b=B))
            mx = tailp.tile([B, 1], F32)
            nc.vector.tensor_reduce(mx[:], Y4[:], mybir.AxisListType.X, OP.max)
            nmx = tailp.tile([B, 1], F32)
            nc.vector.tensor_scalar(nmx[:], mx[:], -1.0, None, OP.mult)
            ex = tailp.tile([B, S], F32)
            sm = tailp.tile([B, 1], F32)
            nc.scalar.activation(ex[:], Y4[:], AF.Exp, bias=nmx[:],
                                 accum_out=sm[:])
            ln = tailp.tile([B, 1], F32)
            nc.scalar.activation(ln[:], sm[:], AF.Ln)
            tot = tailp.tile([B, 1], F32)
            nc.vector.tensor_tensor(tot[:], mx[:], ln[:], OP.add)
            nc.vector.tensor_scalar(ex[:], Y4[:], tot[:], None, OP.subtract)
            nc.sync.dma_start(out_d[:], ex[:])
            tailp_cm.__exit__(None, None, None)

    if finalize:
        nc.finalize()
    return nc


# ---------------------------------------------------------------------------
# host-side wrapper
# ---------------------------------------------------------------------------

def make_in_maps(cfg: Cfg, x, Wg, W1, b1, W2, b2):
    T, D, H, E = cfg.T, cfg.D, cfg.H, cfg.E
    tokens = np.ascontiguousarray(x.reshape(T, D).astype(np.float32))
    tv = tokens.view(np.uint16).reshape(T, D, 2)
    x_lo = np.ascontiguousarray(tv[:, :, 0])
    x_hi = np.ascontiguousarray(tv[:, :, 1])

    iota = np.arange(128, dtype=np.float32)
    cU = (iota[:, None] < iota[None, :]).astype(np.float32)
    cI = np.eye(128, dtype=np.float32)
    cOnesK1 = np.ones((1, 128), np.float32)
    cOnesM1 = np.ones((128, 1), np.float32)
    cREP = (np.arange(16)[:, None] == (np.arange(128)[None, :] % 16)
            ).astype(np.float32)
    cIota8 = np.tile(np.arange(8, dtype=np.float32), (128, 1))
    cIota8m = cIota8 - 999.0
    cIota16 = np.tile(np.arange(16, dtype=np.float32), (128, 1))
    cIota128 = np.tile(iota, (128, 1))
    cIotaTHI = np.tile(np.arange(cfg.THI, dtype=np.float32), (128, 1))
    # cTval[p, j] = global token id of sequence slot s = j*128 + p
    jj = np.arange(cfg.J)
    cTval = ((jj[None, :] % cfg.JT) * 128 +
             np.arange(128)[:, None]).astype(np.float32)

    common = dict(x_lo=x_lo, x_hi=x_hi, wg=np.ascontiguousarray(Wg, np.float32),
                  cU=cU, cI=cI, cOnesK1=cOnesK1, cOnesM1=cOnesM1, cREP=cREP,
                  cIota8=cIota8, cIota8m=cIota8m, cIota16=cIota16,
                  cIota128=cIota128, cIotaTHI=cIotaTHI, cTval=cTval)

    in_maps = []
    for r in range(cfg.NC):
        # wrapped int16 token indices for this core's logits gather
        i = np.arange(cfg.TL)
        w16 = np.zeros((16, cfg.TL // 16), np.int16)
        w16[i % 16, i // 16] = r * cfg.TL + i
        w = np.tile(w16, (8, 1))
        m = dict(common)
        m.update(
            w1e=np.ascontiguousarray(W1[r], np.float32),
            b1e=np.ascontiguousarray(
                b1[r].reshape(cfg.HT, 128).T, np.float32),
            w2e=np.ascontiguousarray(W2[r], np.float32),
            b2e=np.ascontiguousarray(b2[r].reshape(1, -1), np.float32),
            eid=np.full((128, 1), float(r), np.float32),
            tokidx=np.ascontiguousarray(w),
        )
        in_maps.append(m)
    return in_maps


_PROGRAM_CACHE = {}


def _get_program(cfg: Cfg):
    if cfg not in _PROGRAM_CACHE:
        _PROGRAM_CACHE[cfg] = build_program(cfg)
    return _PROGRAM_CACHE[cfg]


def kernel(x, Wg, W1, b1, W2, b2):
    cfg = Cfg(B=x.shape[0], S=x.shape[1], D=x.shape[2], H=W1.shape[2],
              E=Wg.shape[1], K=2, NC=8)
    nc = _get_program(cfg)
    in_maps = make_in_maps(cfg, np.asarray(x), np.asarray(Wg), np.asarray(W1),
                           np.asarray(b1), np.asarray(W2), np.asarray(b2))
    res = run_bass_kernel_spmd(nc, in_maps, core_ids=list(range(cfg.NC)))
    return np.asarray(res.results[0]["out"], dtype=np.float32)


# revision 19
# speedup vs baseline: 1.0858x; 1.0858x over previous
"""MoE routing kernel for Trainium2 (8 NeuronCores, expert-parallel).

Computes, for full inputs x[B,S,D], Wg[D,E], W1[E,D,H], b1[E,H], W2[E,H,D],
b2[E,D]:
    y = moe_forward(x, ...)            # GShard/tutel style top-2 capacity MoE
    out = log_softmax(sum(y, axis=D), axis=S)   # [B, S]

Key algebraic simplification: only row-sums of the second expert matmul are
needed, so  sum_d(yexp[e,c,:]) = relu(x W1[e] + b1[e]) . rowsum(W2[e]) +
sum(b2[e]).  The rowsum of W2 is computed on device.

Distribution: expert-parallel (core r owns expert r), data-parallel router,
replicated position scan, AllGather for logits, AllReduce for the final
per-token scalars.
"""

import sys

for _p in ("/opt/trn_rl_repo",):
    if _p not in sys.path:
        sys.path.insert(0, _p)

import numpy as np
from dataclasses import dataclass

import concourse.bass as bass
import concourse.tile as tile
from concourse import bacc, mybir, library_config
from concourse.bass_utils import run_bass_kernel_spmd

AF = mybir.ActivationFunctionType
OP = mybir.AluOpType
F32 = mybir.dt.float32
F32R = mybir.dt.float32r
I16 = mybir.dt.int16
U16 = mybir.dt.uint16
I32 = mybir.dt.int32
U32 = mybir.dt.uint32


@dataclass(frozen=True)
class Cfg:
    B: int = 4
    S: int = 2048
    D: int = 2048
    H: int = 2048
    E: int = 8
    K: int = 2          # top-k
    NC: int = 8         # cores

    @property
    def T(self):
        return self.B * self.S

    @property
    def C(self):
        return (self.K * self.T) // self.E

    @property
    def TL(self):  # tokens per core (router data-parallel)
        return self.T // self.NC

    @property
    def DT(self):
        return self.D // 128

    @property
    def HT(self):
        return self.H // 128

    @property
    def J(self):  # number of 128-slot tiles in the k*T priority sequence
        return (self.K * self.T) // 128

    @property
    def JT(self):  # number of 128-token tiles
        return self.T // 128

    @property
    def THI(self):  # t_hi factor for the combine scatter (t = t_hi*128 + t_lo)
        return self.T // 128

    @property
    def R(self):  # c = rr*16 + q factorization, rr in [0, R)
        return self.C // 16

    @property
    def CCH(self):  # dispatch gather chunk (slots)
        return min(256, self.C)

    @property
    def NCC(self):
        return self.C // self.CCH

    @property
    def LCH(self):  # logits gather chunk (tokens)
        return min(256, self.TL)

    @property
    def NLC(self):
        return self.TL // self.LCH


def _ckd(cfg: Cfg):
    assert cfg.T % 128 == 0 and cfg.D % 128 == 0 and cfg.H % 128 == 0
    assert cfg.K == 2 and cfg.E <= 8
    assert cfg.J <= 128 and cfg.R <= 128 and cfg.THI <= 128
    assert cfg.C % 16 == 0 and cfg.TL % cfg.LCH == 0
    assert cfg.D % 256 == 0  # dma_gather elem_size_bytes % 256


def build_program(cfg: Cfg, debug_outputs: bool = False, finalize: bool = True):
    """Build the SPMD bass program (same program for all 8 cores)."""
    _ckd(cfg)
    nc = bacc.Bacc("TRN2", target_bir_lowering=False, debug=False,
                   num_devices=cfg.NC, num_swdge_queues=4)

    T, D, H, E, C = cfg.T, cfg.D, cfg.H, cfg.E, cfg.C
    DT, HT, J, R, THI = cfg.DT, cfg.HT, cfg.J, cfg.R, cfg.THI
    NPOS = J * E

    # ---------------- dram I/O ----------------
    x_lo = nc.dram_tensor("x_lo", [T, D], U16, kind="ExternalInput")
    x_hi = nc.dram_tensor("x_hi", [T, D], U16, kind="ExternalInput")
    wg_d = nc.dram_tensor("wg", [D, E], F32, kind="ExternalInput")
    w1_d = nc.dram_tensor("w1e", [D, H], F32, kind="ExternalInput")
    b1_d = nc.dram_tensor("b1e", [128, HT], F32, kind="ExternalInput")
    w2_d = nc.dram_tensor("w2e", [H, D], F32, kind="ExternalInput")
    b2_d = nc.dram_tensor("b2e", [1, D], F32, kind="ExternalInput")
    eid_d = nc.dram_tensor("eid", [128, 1], F32, kind="ExternalInput")
    tokidx_d = nc.dram_tensor("tokidx", [128, cfg.TL // 16], I16,
                              kind="ExternalInput")
    # constants
    cU_d = nc.dram_tensor("cU", [128, 128], F32, kind="ExternalInput")
    cI_d = nc.dram_tensor("cI", [128, 128], F32, kind="ExternalInput")
    cOnesK1_d = nc.dram_tensor("cOnesK1", [1, 128], F32, kind="ExternalInput")
    cOnesM1_d = nc.dram_tensor("cOnesM1", [128, 1], F32, kind="ExternalInput")
    cREP_d = nc.dram_tensor("cREP", [16, 128], F32, kind="ExternalInput")
    cIota8_d = nc.dram_tensor("cIota8", [128, 8], F32, kind="ExternalInput")
    cIota8m_d = nc.dram_tensor("cIota8m", [128, 8], F32, kind="ExternalInput")
    cIota16_d = nc.dram_tensor("cIota16", [128, 16], F32, kind="ExternalInput")
    cIota128_d = nc.dram_tensor("cIota128", [128, 128], F32, kind="ExternalInput")
    cIotaTHI_d = nc.dram_tensor("cIotaTHI", [128, THI], F32, kind="ExternalInput")
    cTval_d = nc.dram_tensor("cTval", [128, J], F32, kind="ExternalInput")

    out_d = nc.dram_tensor("out", [cfg.B, cfg.S], F32, kind="ExternalOutput")
    dbg = {}
    if debug_outputs:
        dbg["logits"] = nc.dram_tensor("dbg_logits", [128, cfg.JT * E], F32,
                                       kind="ExternalOutput")
        dbg["pos"] = nc.dram_tensor("dbg_pos", [128, J], F32,
                                    kind="ExternalOutput")
        dbg["gts"] = nc.dram_tensor("dbg_gts", [128, J], F32,
                                    kind="ExternalOutput")
        dbg["tok"] = nc.dram_tensor("dbg_tok", [16, R], F32,
                                    kind="ExternalOutput")
        dbg["gate"] = nc.dram_tensor("dbg_gate", [16, R], F32,
                                     kind="ExternalOutput")
        dbg["s"] = nc.dram_tensor("dbg_s", [1, C], F32, kind="ExternalOutput")
        dbg["ysum"] = nc.dram_tensor("dbg_ysum", [THI, 128], F32,
                                     kind="ExternalOutput")

    # collective bounce buffers
    lg_bounce = nc.dram_tensor("lg_bounce", [cfg.TL, E], F32)
    lg_all = nc.dram_tensor("lg_all", [T, E], F32, addr_space="Shared")
    y_bounce = nc.dram_tensor("y_bounce", [THI, 128], F32)
    y_all = nc.dram_tensor("y_all", [THI, 128], F32, addr_space="Shared")

    groups = [list(range(cfg.NC))]

    def r32(ap):
        return ap.bitcast(F32R)

    with tile.TileContext(nc) as tc:
        nc.gpsimd.load_library(library_config.mlp)

        # ---------- persistent pools ----------
        with tc.tile_pool(name="consts", bufs=1) as cpool, \
             tc.tile_pool(name="w1", bufs=1) as w1pool, \
             tc.tile_pool(name="small", bufs=1) as spool:

            # constants to SBUF
            cU = cpool.tile([128, 128], F32)
            cUr = cpool.tile([128, 128], F32R)
            cOnesM1r = cpool.tile([128, 1], F32R)
            cI = cpool.tile([128, 128], F32)
            cOnesK1 = cpool.tile([1, 128], F32)
            cOnesM1 = cpool.tile([128, 1], F32)
            cREP = cpool.tile([16, 128], F32)
            cIota8 = cpool.tile([128, 8], F32)
            cIota8m = cpool.tile([128, 8], F32)
            cIota16 = cpool.tile([128, 16], F32)
            cIota128 = cpool.tile([128, 128], F32)
            cIotaTHI = cpool.tile([128, THI], F32)
            cTval = cpool.tile([128, J], F32)
            nc.scalar.dma_start(cUr[:], cU_d[:].bitcast(F32R))
            nc.scalar.dma_start(cOnesM1r[:], cOnesM1_d[:].bitcast(F32R))
            for sb, dr in ((cU, cU_d), (cI, cI_d), (cOnesK1, cOnesK1_d),
                           (cOnesM1, cOnesM1_d), (cREP, cREP_d),
                           (cIota8, cIota8_d), (cIota8m, cIota8m_d),
                           (cIota16, cIota16_d), (cIota128, cIota128_d),
                           (cIotaTHI, cIotaTHI_d), (cTval, cTval_d)):
                nc.scalar.dma_start(sb[:], dr[:])

            eid = cpool.tile([128, 1], F32)
            nc.scalar.dma_start(eid[:], eid_d[:])
            tokidx = cpool.tile([128, cfg.TL // 16], I16)
            nc.scalar.dma_start(tokidx[:], tokidx_d[:])

            wg_sb = cpool.tile([128, DT, E], F32R)
            nc.scalar.dma_start(
                wg_sb[:],
                wg_d.rearrange("(dt p) e -> p dt e", p=128).bitcast(F32R))

            # ================= P1: router logits (my TL tokens) ============
            # lgT_sb[e, t_local]
            lgp0_cm = tc.tile_pool(name="lgp0", bufs=1)
            lgp0 = lgp0_cm.__enter__()
            lgT = lgp0.tile([8, cfg.TL], F32)
            with tc.tile_pool(name="lgg", bufs=2) as lgp, \
                 tc.tile_pool(name="lgps", bufs=2, space="PSUM") as lgps:
                for lc in range(cfg.NLC):
                    pl_lo = lgp.tile([128, DT, cfg.LCH], U16, tag="pl_lo")
                    pl_hi = lgp.tile([128, DT, cfg.LCH], U16, tag="pl_hi")
                    idxs = tokidx[:, (lc * cfg.LCH) // 16:
                                  ((lc + 1) * cfg.LCH) // 16]
                    nc.gpsimd.dma_gather(pl_lo[:], x_lo[:], idxs, cfg.LCH,
                                         cfg.LCH, D, transpose=True,
                                         queue_num=0)
                    nc.gpsimd.dma_gather(pl_hi[:], x_hi[:], idxs, cfg.LCH,
                                         cfg.LCH, D, transpose=True,
                                         queue_num=1)
                    # recombine fp32 = (hi << 16) | lo
                    xt = lgp.tile([128, DT, cfg.LCH], F32R, tag="xt")
                    for dt in range(DT):
                        stg = lgp.tile([128, cfg.LCH], U32, tag="stg",
                                       bufs=2)
                        nc.vector.scalar_tensor_tensor(
                            stg[:], pl_hi[:, dt, :], 65536, pl_lo[:, dt, :],
                            OP.mult, OP.add)
                        nc.vector.tensor_copy(xt[:, dt, :],
                                              stg.bitcast(F32)[:])
                    ps = lgps.tile([8, cfg.LCH], F32, tag="lgps")
                    for dt in range(DT):
                        nc.tensor.matmul(ps[:], wg_sb[:, dt, :],
                                         xt[:, dt, :],
                                         start=(dt == 0), stop=(dt == DT - 1))
                    nc.scalar.copy(lgT[:, lc * cfg.LCH:(lc + 1) * cfg.LCH],
                                   ps[:])

            # transpose to [t_local % 128, jj, e] then DMA out + AllGather
            lg_me = lgp0.tile([128, cfg.TL // 128, 8], F32)
            with tc.tile_pool(name="lgtp", bufs=2, space="PSUM") as lgtp:
                for m in range(cfg.TL // 128):
                    pst = lgtp.tile([128, 8], F32, tag="pst")
                    nc.tensor.transpose(pst[:], lgT[0:8, m * 128:(m + 1) * 128],
                                        cI[0:8, 0:8])
                    nc.scalar.copy(lg_me[:, m, :], pst[:])
            nc.sync.dma_start(
                lg_bounce.rearrange("(m p) e -> p m e", p=128), lg_me[:])
            nc.gpsimd.collective_compute(
                "AllGather", OP.bypass, replica_groups=groups,
                ins=[lg_bounce[:]], outs=[lg_all[:]])
            lgp0_cm.__exit__(None, None, None)

            # ---- weight loads + W2 rowsum (emitted after router so the
            # router's gathers aren't starved of DMA bandwidth) ----
            b1_sb = spool.tile([128, HT], F32)
            nc.scalar.dma_start(b1_sb[:], b1_d[:])
            w1_sb = w1pool.tile([128, DT, H], F32R)
            for dt in range(DT):
                nc.sync.dma_start(w1_sb[:, dt, :],
                                  w1_d[dt * 128:(dt + 1) * 128, :].bitcast(F32R))
            # w2sum[h] = sum_d W2[h, d]  -> [p, ht], reduced on ACT (accum)
            w2s_sb = spool.tile([128, HT], F32R)
            with tc.tile_pool(name="w2red", bufs=2) as w2pool:
                for ht in range(HT):
                    w2t = w2pool.tile([128, D], F32, tag="w2t")
                    nc.sync.dma_start(w2t[:], w2_d[ht * 128:(ht + 1) * 128, :])
                    with nc.allow_low_precision(reason="fp32r rowsum"):
                        nc.scalar.activation(w2t[:], w2t[:], AF.Copy,
                                             accum_out=w2s_sb[:, ht:ht + 1])
            b2s = spool.tile([1, 1], F32)
            with tc.tile_pool(name="b2red", bufs=1) as b2pool:
                b2row = b2pool.tile([1, D], F32)
                nc.scalar.dma_start(b2row[:], b2_d[:])
                nc.vector.tensor_reduce(b2s[:], b2row[:],
                                        mybir.AxisListType.X, OP.add)

            rtp_cm = tc.tile_pool(name="rtp", bufs=1)
            rtp = rtp_cm.__enter__()
            # LG[p, jt, e] for all T tokens
            LG = rtp.tile([128, cfg.JT, E], F32)
            nc.sync.dma_start(LG[:],
                                lg_all.rearrange("(jt p) e -> p jt e", p=128))
            if debug_outputs:
                nc.sync.dma_start(dbg["logits"][:],
                                    LG[:].rearrange("p a b -> p (a b)"))

            # ================= P2: top-2 + gates ============
            JT = cfg.JT
            m1 = rtp.tile([128, JT], F32)
            nc.vector.tensor_reduce(m1[:], LG[:], mybir.AxisListType.X, OP.max)
            eq1 = rtp.tile([128, JT, E], F32)
            nc.vector.tensor_tensor(
                eq1[:], LG[:],
                m1[:].unsqueeze(2).broadcast_to([128, JT, E]),
                OP.is_equal)
            zt = rtp.tile([128, JT, E], F32)
            nc.vector.tensor_tensor(
                zt[:], eq1[:],
                cIota8m[:].unsqueeze(1).broadcast_to([128, JT, E]),
                OP.mult)
            idx_all = rtp.tile([128, J], F32)
            i1m = rtp.tile([128, JT], F32)
            nc.vector.tensor_reduce(i1m[:], zt[:], mybir.AxisListType.X, OP.min)
            nc.vector.tensor_scalar(idx_all[:, 0:JT], i1m[:], 999.0, None,
                                    OP.add)
            masked = rtp.tile([128, JT, E], F32)
            nc.vector.scalar_tensor_tensor(masked[:], eq1[:], -1e30, LG[:],
                                           OP.mult, OP.add)
            m2 = rtp.tile([128, JT], F32)
            nc.vector.tensor_reduce(m2[:], masked[:], mybir.AxisListType.X,
                                    OP.max)
            eq2 = rtp.tile([128, JT, E], F32)
            nc.vector.tensor_tensor(
                eq2[:], masked[:],
                m2[:].unsqueeze(2).broadcast_to([128, JT, E]),
                OP.is_equal)
            zt2 = rtp.tile([128, JT, E], F32)
            nc.vector.tensor_tensor(
                zt2[:], eq2[:],
                cIota8m[:].unsqueeze(1).broadcast_to([128, JT, E]),
                OP.mult)
            i2m = rtp.tile([128, JT], F32)
            nc.vector.tensor_reduce(i2m[:], zt2[:], mybir.AxisListType.X,
                                    OP.min)
            nc.vector.tensor_scalar(idx_all[:, JT:J], i2m[:], 999.0, None,
                                    OP.add)
            # gates: g1 = sigmoid(m1 - m2), g2 = 1 - g1
            gts = rtp.tile([128, J], F32)
            d12 = rtp.tile([128, JT], F32)
            nc.vector.tensor_tensor(d12[:], m1[:], m2[:], OP.subtract)
            nc.scalar.activation(gts[:, 0:JT], d12[:], AF.Sigmoid)
            nc.vector.tensor_scalar(gts[:, JT:J], gts[:, 0:JT], -1.0, 1.0,
                                    OP.mult, OP.add)

            # ================= P3: position scan ============
            OH = rtp.tile([128, J, E], F32R)
            nc.vector.tensor_tensor(
                OH[:],
                idx_all[:].unsqueeze(2).broadcast_to([128, J, E]),
                cIota8[:].unsqueeze(1).broadcast_to([128, J, E]),
                OP.is_equal)
            OHf32 = OH.bitcast(F32)
            OHf = OH[:].rearrange("p a b -> p (a b)")
            nch = (NPOS + 511) // 512
            csz = NPOS // nch
            scanps_cm = tc.tile_pool(name="scanps", bufs=1, space="PSUM")
            scanps = scanps_cm.__enter__()
            ps_pos = scanps.tile([128, NPOS], F32)
            ps_tot = scanps.tile([1, NPOS], F32)
            for h in range(nch):
                sl = slice(h * csz, (h + 1) * csz)
                nc.tensor.matmul(ps_pos[:, sl], cUr[:], OHf[:, sl],
                                 start=True, stop=True)
                nc.tensor.matmul(ps_tot[:, sl], cOnesM1r[:],
                                 OHf[:, sl], start=True, stop=True)
            tot_row = rtp.tile([1, NPOS], F32)
            nc.scalar.copy(tot_row[:], ps_tot[:])
            tot_jt = rtp.tile([J, E], F32)
            nc.sync.dma_start(tot_jt[:], tot_row[:])
            ps_off = scanps.tile([J, E], F32)
            nc.tensor.matmul(ps_off[:], cU[0:J, 0:J], tot_jt[:],
                             start=True, stop=True)
            off_sb = rtp.tile([J, E], F32)
            nc.scalar.copy(off_sb[:], ps_off[:])
            off_row = rtp.tile([1, NPOS], F32)
            nc.sync.dma_start(off_row[:], off_sb[:])
            ps_bc = scanps.tile([128, NPOS], F32)
            for h in range(nch):
                sl = slice(h * csz, (h + 1) * csz)
                nc.tensor.matmul(ps_bc[:, sl], cOnesK1[:], off_row[:, sl],
                                 start=True, stop=True)
            # pos_full = intra + offset broadcast
            posf = rtp.tile([128, NPOS], F32)
            nc.scalar.copy(posf[:], ps_pos[:])
            nc.vector.tensor_tensor(posf[:], posf[:], ps_bc[:], OP.add)
            # select position of chosen expert; keep; final gates
            tmp = rtp.tile([128, J, E], F32)
            nc.vector.tensor_tensor(tmp[:].rearrange("p a b -> p (a b)"),
                                    OHf32[:].rearrange("p a b -> p (a b)"),
                                    posf[:], OP.mult)
            POS = rtp.tile([128, J], F32)
            nc.vector.tensor_reduce(POS[:], tmp[:], mybir.AxisListType.X,
                                    OP.add)
            scanps_cm.__exit__(None, None, None)
            KEEP = rtp.tile([128, J], F32)
            nc.vector.tensor_scalar(KEEP[:], POS[:], float(C), None, OP.is_lt)
            gfin = rtp.tile([128, J], F32)
            nc.vector.tensor_tensor(gfin[:], gts[:], KEEP[:], OP.mult)
            if debug_outputs:
                nc.sync.dma_start(dbg["pos"][:], POS[:])
                nc.sync.dma_start(dbg["gts"][:], gfin[:])

            # ================= P4: inversion (slot -> token, gate) =========
            SEL = rtp.tile([128, J], F32)
            nc.vector.tensor_scalar(SEL[:], idx_all[:], eid[:], None,
                                    OP.is_equal)
            selk = rtp.tile([128, J], F32)
            nc.vector.tensor_tensor(selk[:], SEL[:], KEEP[:], OP.mult)
            valt = rtp.tile([128, J], F32)
            nc.vector.tensor_tensor(valt[:], cTval[:, 0:J], selk[:], OP.mult)
            valg = rtp.tile([128, J], F32)
            nc.vector.tensor_tensor(valg[:], gfin[:], selk[:], OP.mult)
            # integer split of POS: q = POS & 15, rr = POS >> 4
            posi = rtp.tile([128, J], I32)
            nc.vector.tensor_copy(posi[:], POS[:])
            rri = rtp.tile([128, J], I32)
            nc.vector.tensor_scalar(rri[:], posi[:], 4, None,
                                    OP.logical_shift_right)
            qi = rtp.tile([128, J], I32)
            nc.vector.tensor_scalar(qi[:], posi[:], 15, None, OP.bitwise_and)
            rrf = rtp.tile([128, J], F32)
            nc.vector.tensor_copy(rrf[:], rri[:])
            qf = rtp.tile([128, J], F32)
            nc.vector.tensor_copy(qf[:], qi[:])

            invps_cm = tc.tile_pool(name="invps", bufs=1, space="PSUM")
            invps = invps_cm.__enter__()
            ps_inv = invps.tile([32, R], F32)
            STR = 4  # j's per strip
            with tc.tile_pool(name="invp", bufs=2) as invp:
                for s0 in range(0, J, STR):
                    js = slice(s0, s0 + STR)
                    aoh = invp.tile([128, STR, 16], F32, tag="aoh")
                    nc.vector.tensor_tensor(
                        aoh[:],
                        qf[:, js].unsqueeze(2).broadcast_to(
                            [128, STR, 16]),
                        cIota16[:].unsqueeze(1).broadcast_to(
                            [128, STR, 16]),
                        OP.is_equal)
                    atile = invp.tile([128, STR, 32], F32, tag="atile")
                    nc.vector.tensor_tensor(
                        atile[:, :, 0:16], aoh[:],
                        valt[:, js].unsqueeze(2).broadcast_to(
                            [128, STR, 16]),
                        OP.mult)
                    nc.vector.tensor_tensor(
                        atile[:, :, 16:32], aoh[:],
                        valg[:, js].unsqueeze(2).broadcast_to(
                            [128, STR, 16]),
                        OP.mult)
                    btile = invp.tile([128, STR, R], F32, tag="btile")
                    nc.vector.tensor_tensor(
                        btile[:],
                        rrf[:, js].unsqueeze(2).broadcast_to(
                            [128, STR, R]),
                        cIota128[:, 0:R].unsqueeze(1).broadcast_to(
                            [128, STR, R]),
                        OP.is_equal)
                    for jj in range(STR):
                        j = s0 + jj
                        nc.tensor.matmul(ps_inv[:], atile[:, jj, :],
                                         btile[:, jj, :],
                                         start=(j == 0), stop=(j == J - 1))
            rtp_cm.__exit__(None, None, None)
            inv_sb = spool.tile([32, R], F32)
            nc.scalar.copy(inv_sb[:], ps_inv[:])
            if debug_outputs:
                nc.sync.dma_start(dbg["tok"][:], inv_sb[0:16, :])
                nc.sync.dma_start(dbg["gate"][:], inv_sb[16:32, :])
            # gate map to partitions 0:16 (cross-partition -> DMA)
            ge16 = spool.tile([16, R], F32)
            nc.sync.dma_start(ge16[:], inv_sb[16:32, :])
            # replicate token map over 128 partitions, convert to int16
            ps_rep = invps.tile([128, R], F32, tag="ps_rep")
            nc.tensor.matmul(ps_rep[:], cREP[:], inv_sb[0:16, :],
                             start=True, stop=True)
            tokmap16 = spool.tile([128, R], I16)
            nc.vector.tensor_copy(tokmap16[:], ps_rep[:])
            invps_cm.__exit__(None, None, None)

            # ================= P5: dispatch gather + expert FFN ============
            sT = spool.tile([R, 16], F32)
            CCH = cfg.CCH
            with tc.tile_pool(name="ffg", bufs=1) as gpl, \
                 tc.tile_pool(name="ffx", bufs=2) as xpl, \
                 tc.tile_pool(name="ffh", bufs=2) as hpl, \
                 tc.tile_pool(name="ffps", bufs=2, space="PSUM") as fps, \
                 tc.tile_pool(name="ffss", bufs=2, space="PSUM") as sps:
                for cc in range(cfg.NCC):
                    idxs = tokmap16[:, (cc * CCH) // 16:((cc + 1) * CCH) // 16]
                    g_lo = gpl.tile([128, DT, CCH], U16, tag="g_lo")
                    g_hi = gpl.tile([128, DT, CCH], U16, tag="g_hi")
                    nc.gpsimd.dma_gather(g_lo[:], x_lo[:], idxs, CCH, CCH, D,
                                         transpose=True, queue_num=2)
                    nc.gpsimd.dma_gather(g_hi[:], x_hi[:], idxs, CCH, CCH, D,
                                         transpose=True, queue_num=3)
                    xet = xpl.tile([128, DT, CCH], F32R, tag="xet")
                    for dt in range(DT):
                        fstg = xpl.tile([128, CCH], U32, tag="fstg", bufs=2)
                        nc.vector.scalar_tensor_tensor(
                            fstg[:], g_hi[:, dt, :], 65536, g_lo[:, dt, :],
                            OP.mult, OP.add)
                        nc.vector.tensor_copy(xet[:, dt, :],
                                              fstg.bitcast(F32)[:])
                    ps_s = sps.tile([1, CCH], F32, tag="ps_s")
                    for ht in range(HT):
                        ps_h = fps.tile([128, CCH], F32, tag="ps_h")
                        for dt in range(DT):
                            nc.tensor.matmul(
                                ps_h[:],
                                w1_sb[:, dt, ht * 128:(ht + 1) * 128],
                                xet[:, dt, :],
                                start=(dt == 0), stop=(dt == DT - 1))
                        ht_sb = hpl.tile([128, CCH], F32R, tag="ht_sb")
                        nc.scalar.activation(ht_sb[:], ps_h[:], AF.Relu,
                                             bias=b1_sb[:, ht:ht + 1])
                        nc.tensor.matmul(ps_s[:], w2s_sb[:, ht:ht + 1],
                                         ht_sb[:],
                                         start=(ht == 0), stop=(ht == HT - 1))
                    s_st = hpl.tile([1, CCH], F32, tag="s_st")
                    nc.vector.tensor_scalar(s_st[:], ps_s[:], b2s[0:1, 0:1],
                                            None, OP.add)
                    nc.scalar.dma_start(
                        sT[(cc * CCH) // 16:((cc + 1) * CCH) // 16, :],
                        s_st[:])
            if debug_outputs:
                nc.sync.dma_start(dbg["s"][:], sT[:])

            # ================= P6: combine scatter ============
            combps_cm = tc.tile_pool(name="combps", bufs=1, space="PSUM")
            combps = combps_cm.__enter__()
            ps_geT = combps.tile([128, 16], F32, tag="ps_geT")
            nc.tensor.transpose(ps_geT[0:R, :], ge16[:], cI[0:16, 0:16])
            wT = spool.tile([R, 16], F32)
            nc.vector.tensor_tensor(wT[:], sT[:], ps_geT[0:R, :], OP.mult)
            ps_tT = combps.tile([128, 16], F32, tag="ps_tT")
            nc.tensor.transpose(ps_tT[0:R, :], inv_sb[0:16, :], cI[0:16, 0:16])
            tokT = spool.tile([R, 16], F32)
            nc.scalar.copy(tokT[:], ps_tT[0:R, :])
            # integer split: thi = tok >> 7, tlo = tok & 127
            tki = spool.tile([R, 16], I32)
            nc.vector.tensor_copy(tki[:], tokT[:])
            thi_i = spool.tile([R, 16], I32)
            nc.vector.tensor_scalar(thi_i[:], tki[:], 7, None,
                                    OP.logical_shift_right)
            tlo_i = spool.tile([R, 16], I32)
            nc.vector.tensor_scalar(tlo_i[:], tki[:], 127, None,
                                    OP.bitwise_and)
            thi_f = spool.tile([R, 16], F32)
            nc.vector.tensor_copy(thi_f[:], thi_i[:])
            tlo_f = spool.tile([R, 16], F32)
            nc.vector.tensor_copy(tlo_f[:], tlo_i[:])

            ps_y = combps.tile([THI, 128], F32)
            with tc.tile_pool(name="scat", bufs=2) as scp:
                for j in range(16):
                    a_j = scp.tile([R, THI], F32, tag="a_j")
                    nc.vector.scalar_tensor_tensor(
                        a_j[:], cIotaTHI[0:R, :], thi_f[:, j:j + 1],
                        wT[:, j:j + 1].broadcast_to([R, THI]),
                        OP.is_equal, OP.mult)
                    b_j = scp.tile([R, 128], F32, tag="b_j")
                    nc.vector.tensor_scalar(b_j[:], cIota128[0:R, :],
                                            tlo_f[:, j:j + 1], None,
                                            OP.is_equal)
                    nc.tensor.matmul(ps_y[:], a_j[:], b_j[:],
                                     start=(j == 0), stop=(j == 15))
            ysum_sb = spool.tile([THI, 128], F32)
            nc.scalar.copy(ysum_sb[:], ps_y[:])
            combps_cm.__exit__(None, None, None)
            if debug_outputs:
                nc.sync.dma_start(dbg["ysum"][:], ysum_sb[:])
            nc.sync.dma_start(y_bounce[:], ysum_sb[:])
            nc.gpsimd.collective_compute(
                "AllReduce", OP.add, replica_groups=groups,
                ins=[y_bounce[:]], outs=[y_all[:]])

            # ================= P7: log_softmax over S ============
            B, S = cfg.B, cfg.S
            tailp_cm = tc.tile_pool(name="tailp", bufs=1)
            tailp = tailp_cm.__enter__()
            Y4 = tailp.tile([B, S], F32)
            nc.sync.dma_start(Y4[:],
                                y_all.rearrange("(b u) l -> b (u l)", b=B))
            mx = tailp.tile([B, 1], F32)
            nc.vector.tensor_reduce(mx[:], Y4[:], mybir.AxisListType.X, OP.max)
            nmx = tailp.tile([B, 1], F32)
            nc.vector.tensor_scalar(nmx[:], mx[:], -1.0, None, OP.mult)
            ex = tailp.tile([B, S], F32)
            sm = tailp.tile([B, 1], F32)
            nc.scalar.activation(ex[:], Y4[:], AF.Exp, bias=nmx[:],
                                 accum_out=sm[:])
            ln = tailp.tile([B, 1], F32)
            nc.scalar.activation(ln[:], sm[:], AF.Ln)
            tot = tailp.tile([B, 1], F32)
            nc.vector.tensor_tensor(tot[:], mx[:], ln[:], OP.add)
            nc.vector.tensor_scalar(ex[:], Y4[:], tot[:], None, OP.subtract)
            nc.sync.dma_start(out_d[:], ex[:])
            tailp_cm.__exit__(None, None, None)

    if finalize:
        nc.finalize()
    return nc


# ---------------------------------------------------------------------------
# host-side wrapper
# ---------------------------------------------------------------------------

def make_in_maps(cfg: Cfg, x, Wg, W1, b1, W2, b2):
    T, D, H, E = cfg.T, cfg.D, cfg.H, cfg.E
    tokens = np.ascontiguousarray(x.reshape(T, D).astype(np.float32))
    tv = tokens.view(np.uint16).reshape(T, D, 2)
    x_lo = np.ascontiguousarray(tv[:, :, 0])
    x_hi = np.ascontiguousarray(tv[:, :, 1])

    iota = np.arange(128, dtype=np.float32)
    cU = (iota[:, None] < iota[None, :]).astype(np.float32)
    cI = np.eye(128, dtype=np.float32)
    cOnesK1 = np.ones((1, 128), np.float32)
    cOnesM1 = np.ones((128, 1), np.float32)
    cREP = (np.arange(16)[:, None] == (np.arange(128)[None, :] % 16)
            ).astype(np.float32)
    cIota8 = np.tile(np.arange(8, dtype=np.float32), (128, 1))
    cIota8m = cIota8 - 999.0
    cIota16 = np.tile(np.arange(16, dtype=np.float32), (128, 1))
    cIota128 = np.tile(iota, (128, 1))
    cIotaTHI = np.tile(np.arange(cfg.THI, dtype=np.float32), (128, 1))
    # cTval[p, j] = global token id of sequence slot s = j*128 + p
    jj = np.arange(cfg.J)
    cTval = ((jj[None, :] % cfg.JT) * 128 +
             np.arange(128)[:, None]).astype(np.float32)

    common = dict(x_lo=x_lo, x_hi=x_hi, wg=np.ascontiguousarray(Wg, np.float32),
                  cU=cU, cI=cI, cOnesK1=cOnesK1, cOnesM1=cOnesM1, cREP=cREP,
                  cIota8=cIota8, cIota8m=cIota8m, cIota16=cIota16,
                  cIota128=cIota128, cIotaTHI=cIotaTHI, cTval=cTval)

    in_maps = []
    for r in range(cfg.NC):
        # wrapped int16 token indices for this core's logits gather
        i = np.arange(cfg.TL)
        w16 = np.zeros((16, cfg.TL // 16), np.int16)
        w16[i % 16, i // 16] = r * cfg.TL + i
        w = np.tile(w16, (8, 1))
        m = dict(common)
        m.update(
            w1e=np.ascontiguousarray(W1[r], np.float32),
            b1e=np.ascontiguousarray(
                b1[r].reshape(cfg.HT, 128).T, np.float32),
            w2e=np.ascontiguousarray(W2[r], np.float32),
            b2e=np.ascontiguousarray(b2[r].reshape(1, -1), np.float32),
            eid=np.full((128, 1), float(r), np.float32),
            tokidx=np.ascontiguousarray(w),
        )
        in_maps.append(m)
    return in_maps


_PROGRAM_CACHE = {}


def _get_program(cfg: Cfg):
    if cfg not in _PROGRAM_CACHE:
        _PROGRAM_CACHE[cfg] = build_program(cfg)
    return _PROGRAM_CACHE[cfg]


def kernel(x, Wg, W1, b1, W2, b2):
    cfg = Cfg(B=x.shape[0], S=x.shape[1], D=x.shape[2], H=W1.shape[2],
              E=Wg.shape[1], K=2, NC=8)
    nc = _get_program(cfg)
    in_maps = make_in_maps(cfg, np.asarray(x), np.asarray(Wg), np.asarray(W1),
                           np.asarray(b1), np.asarray(W2), np.asarray(b2))
    res = run_bass_kernel_spmd(nc, in_maps, core_ids=list(range(cfg.NC)))
    return np.asarray(res.results[0]["out"], dtype=np.float32)
